# revision 43
# baseline (speedup 1.0000x reference)
"""Cross-frame attention kernel for 8 TRN2 NeuronCores.

Sharding: core c handles batch b = c//2 and head-group g = c%2 (4 of the 8
heads).  The host pre-transposes x[b]/context[b] (feature dim onto SBUF
partitions) and casts to bf16; each core computes a partial output
(its 4 heads pushed through the matching Wo rows) and the host sums the
two partials per batch plus the bias.

Device math per core (S^T layout, softmax over the partition j-dim):
  QT = Wq_g^T x^T          [256, 2048] -> fp8 q8 + fp8 residual qr8
  KT = Wk_g^T c^T          [256, 2048] -> fp8 k8 (stored twice, DR slots)
  V  = c Wv_g              [2048, 256] bf16 (+ ones column per head)
  S^T = K_h Q_h^T          via fp8 DoubleRow matmul: k8.T@(q8+qr8)
  exp via ScalarE (scale=1/8 fused), bf16
  O~^T | Z = [V_h|1]^T expS^T   (PSUM accumulate over j, bf16)
  A^T = O~^T * bcast(1/Z)  (reciprocal_approx_fast + gpsimd broadcast)
  out_partial = A^T^T Wo_g [2048, 512] fp32

Logits are |S/8| <~ 1.1 for this problem's scale, so softmax without
max-subtraction is exact in fp32.  fp8 DoubleRow S with the q-side
residual keeps q at ~16-bit precision; only k's fp8 rounding (~2.4%
elementwise) reaches the logits, measured end-to-end rel err ~5.9e-3.

Note: on this machine fp8/DoubleRow matmuls STREAM no faster than bf16
(~0.74 ns/col at N=512 regardless of dtype), but the fp8 S-path lowers
tensor-engine power enough that the chip stays out of DVFS throttling
(which otherwise halves the PE clock after ~100us of sustained bf16
load) — that is where most of the speedup comes from.
"""

import numpy as np
import ml_dtypes

B = 4
N = 2048  # query length
M = 2048  # context length
DIM = 512
HEADS = 8
DH = 64
HC = 256  # head columns handled per core (4 heads)
P = 128
KO = DIM // P  # 4 k-chunks
NI4 = N // 512  # 4 i-chunks of 512
NJ = M // P  # 16 j-chunks
JPG = 2  # j-chunks per exp group (PSUM banks per S^T buffer)
JF = 0  # leading j-chunks per block on the fp8 DoubleRow O path

_CACHE = {}


def _build():
    from contextlib import ExitStack

    import concourse.mybir as mybir
    import concourse.tile as tile
    from concourse import bacc

    bf = mybir.dt.bfloat16
    f32 = mybir.dt.float32
    f8 = mybir.dt.float8e4
    Exp = mybir.ActivationFunctionType.Exp
    DR = mybir.MatmulPerfMode.DoubleRow
    sub_op = mybir.AluOpType.subtract

    nc = bacc.Bacc(None, target_bir_lowering=False, debug=False)
    with tile.TileContext(nc) as tc:
        with ExitStack() as ctx:
            dram = ctx.enter_context(tc.tile_pool(name="dram", bufs=1, space="DRAM"))
            xT_d = dram.tile([DIM, N], bf, kind="ExternalInput")
            cT_d = dram.tile([DIM, M], bf, kind="ExternalInput")
            wq_d = dram.tile([DIM, HC], bf, kind="ExternalInput")
            wk_d = dram.tile([DIM, HC], bf, kind="ExternalInput")
            wv_d = dram.tile([DIM, HC], bf, kind="ExternalInput")
            wo_d = dram.tile([HC, DIM], bf, kind="ExternalInput")
            out_d = dram.tile([N, DIM], f32, kind="ExternalOutput")

            const = ctx.enter_context(tc.tile_pool(name="const", bufs=1))

            xt_sb = const.tile([P, KO, N], bf, tag="xt")
            ct_sb = const.tile([P, KO, M], bf, tag="ct")
            wq_sb = const.tile([P, KO, HC], bf, tag="wq")
            wk_sb = const.tile([P, KO, HC], bf, tag="wk")
            wv_sb = const.tile([P, KO, HC], bf, tag="wv")
            wo_sb = const.tile([P, 2, DIM], bf, tag="wo")
            # fp8 Q (slot 0) + residual (slot 1); head hl at partitions 64*hl
            q8_sb = const.tile([P, 2, 2, N], f8, tag="q8")
            # fp8 K duplicated into both DoubleRow slots
            k8_sb = const.tile([P, 2, 2, M], f8, tag="k8")
            # all 4 heads' V with a trailing ones column: [j, jo, head, 65]
            vp_sb = const.tile([P, NJ, 4, DH + 1], bf, tag="vp")
            # fp8 copy of the first JF j-tiles for the DoubleRow O path.
            # Padded to 68 columns per (jo, head): the DR ldweights outer
            # step must be 16B-aligned (4 heads x 68 = 272 = 16*17).
            vp8_sb = const.tile([P, max(JF, 1), 4, DH + 4], f8, tag="vp8")
            aT_sb = const.tile([P, 2, N], bf, tag="aT")
            ones_sb = const.tile([1, DH], bf, tag="ones1")

            dummy_sb = const.tile([1, 1], f32, tag="dummy")
            nc.vector.memset(ones_sb[:], 1.0)
            nc.vector.memset(vp_sb[:, :, :, DH : DH + 1], 1.0)
            nc.vector.memset(vp8_sb[:], 0.0)
            nc.vector.memset(vp8_sb[:, :, :, DH : DH + 1], 1.0)
            # hoist the exp ACT-table load out of the critical path
            nc.scalar.activation(dummy_sb[:], ones_sb[0:1, 0:1], Exp, scale=1.0)

            # DMA in dependency order.  The critical path to the first exp is
            # q_proj (wq + xt chunk 0) then a mini k-proj over the first two
            # j-tiles (wk + ct[:, :, 0:256]).  A single 128KB piece takes
            # ~6.5us on one DMA engine, so the chunk-0 pieces are split into
            # 256-col halves and the Q-side (sync queue) and K-side (gpsimd
            # queue) streams issue in parallel.
            cT_r = cT_d[:].rearrange("(ko p) i -> p ko i", p=P)
            xT_r = xT_d[:].rearrange("(ko p) i -> p ko i", p=P)
            nc.sync.dma_start(wq_sb[:], wq_d[:].rearrange("(ko p) m -> p ko m", p=P))
            nc.gpsimd.dma_start(wk_sb[:], wk_d[:].rearrange("(ko p) m -> p ko m", p=P))
            # chunk-0 pieces fan out over three DMA-capable queues (Act is
            # idle here) so issue and transfer both parallelize
            for ko in range(KO):
                nc.sync.dma_start(xt_sb[:, ko, 0:256], xT_r[:, ko, 0:256])
                nc.scalar.dma_start(xt_sb[:, ko, 256:512], xT_r[:, ko, 256:512])
            for ko in range(KO):
                nc.gpsimd.dma_start(ct_sb[:, ko, 0:256], cT_r[:, ko, 0:256])
            for ko in range(KO):
                nc.gpsimd.dma_start(ct_sb[:, ko, 256:512], cT_r[:, ko, 256:512])
            nc.gpsimd.dma_start(wv_sb[:], wv_d[:].rearrange("(ko p) m -> p ko m", p=P))
            for i4 in range(1, NI4):
                isl = slice(i4 * 512, (i4 + 1) * 512)
                for ko in range(KO):
                    nc.gpsimd.dma_start(ct_sb[:, ko, isl], cT_r[:, ko, isl])
            for i4 in range(1, NI4):
                isl = slice(i4 * 512, (i4 + 1) * 512)
                for ko in range(KO):
                    nc.sync.dma_start(xt_sb[:, ko, isl], xT_r[:, ko, isl])
            nc.gpsimd.dma_start(wo_sb[:], wo_d[:].rearrange("(r p) n -> p r n", p=P))

            # Single shared PSUM budget (8 banks):
            #   s-tag 2x2 + aux 2x1 (shared with wo) + o 2x1
            with (
                tc.tile_pool(name="s_ps", bufs=2, space="PSUM") as s_pool,
                tc.tile_pool(name="aux_ps", bufs=2, space="PSUM") as aux_pool,
                tc.tile_pool(name="o_ps", bufs=2, space="PSUM") as o_pool,
                tc.tile_pool(name="e_sb", bufs=4) as e_pool,
                tc.tile_pool(name="small", bufs=2) as small,
                tc.tile_pool(name="ost", bufs=2) as ostp,
            ):
                def k_proj(m, chunk, lo=0, hi=512):
                    isl = slice(chunk * 512 + lo, chunk * 512 + hi)
                    ps = aux_pool.tile([P, 512], f32, tag="aux", name="ps_k")
                    for ko in range(KO):
                        nc.tensor.matmul(
                            ps[:, 0 : hi - lo],
                            wk_sb[:, ko, m * P : (m + 1) * P],
                            ct_sb[:, ko, isl],
                            start=(ko == 0),
                            stop=(ko == KO - 1),
                        )
                    nc.vector.tensor_copy(k8_sb[:, m, 0, isl], ps[:, 0 : hi - lo])
                    nc.vector.tensor_copy(k8_sb[:, m, 1, isl], ps[:, 0 : hi - lo])

                def q_proj(m, chunk):
                    isl = slice(chunk * 512, (chunk + 1) * 512)
                    ps = aux_pool.tile([P, 512], f32, tag="aux", name="ps_q")
                    for ko in range(KO):
                        nc.tensor.matmul(
                            ps[:],
                            wq_sb[:, ko, m * P : (m + 1) * P],
                            xt_sb[:, ko, isl],
                            start=(ko == 0),
                            stop=(ko == KO - 1),
                        )
                    nc.vector.tensor_copy(q8_sb[:, m, 0, isl], ps[:])
                    nc.vector.tensor_tensor(
                        q8_sb[:, m, 1, isl], ps[:], q8_sb[:, m, 0, isl], sub_op
                    )

                def att_block(i4, m, hl, fillers=None, vfill=False):
                    isl = slice(i4 * 512, (i4 + 1) * 512)
                    h = 2 * m + hl
                    pb = DH * hl
                    # 68 partitions: rows 65-67 take the vp8 padding columns
                    # (zeros); Z stays at row DH
                    o_ps = o_pool.tile([DH + 4, 512], f32, tag="o", name="o_ps")
                    for jg in range(NJ // JPG):
                        s_ps = s_pool.tile([P, JPG, 512], f32, tag="s", name="s_ps")
                        for jj in range(JPG):
                            j = jg * JPG + jj
                            nc.tensor.matmul(
                                s_ps[:, jj, :],
                                k8_sb[pb : pb + DH, m, :, j * P : (j + 1) * P],
                                q8_sb[pb : pb + DH, m, :, isl],
                                start=True,
                                stop=True,
                                perf_mode=DR,
                            )
                        dr8 = jg < JF // JPG
                        if dr8:
                            e_sb = e_pool.tile([P, JPG, 512], f8, tag="e8",
                                               name="e8_sb")
                        else:
                            e_sb = e_pool.tile([P, JPG, 512], bf, tag="e",
                                               name="e_sb")
                        nc.scalar.activation(e_sb[:], s_ps[:], Exp, scale=0.125)
                        # projection fillers sit between the S matmuls and the
                        # O accumulation so they overlap the act-engine exp.
                        if vfill:
                            vpair(jg)()
                        for f in (fillers or {}).get(jg, []):
                            f()
                        if dr8:
                            # fp8 DoubleRow: both j-tiles of the group in one
                            # matmul (slots carry the pair) — half the PE time
                            nc.tensor.matmul(
                                o_ps[:],
                                vp8_sb[:, jg * JPG : jg * JPG + JPG, h, :],
                                e_sb[:],
                                start=(jg == 0),
                                stop=False,
                                perf_mode=DR,
                            )
                        else:
                            for jj in range(JPG):
                                j = jg * JPG + jj
                                nc.tensor.matmul(
                                    o_ps[0 : DH + 1, :],
                                    vp_sb[:, j, h, :],
                                    e_sb[:, jj, :],
                                    start=(j == 0),
                                    stop=(j == NJ - 1),
                                )
                    # recip_approx_fast's fp32 bit-trick misreads PSUM on HW:
                    # stage Z into SBUF first.
                    zs = small.tile([1, 512], f32, tag="zs", name="zs")
                    nc.vector.tensor_copy(zs[:], o_ps[DH : DH + 1, :])
                    rz = small.tile([1, 512], f32, tag="rz", name="rz")
                    nc.vector.reciprocal_approx_fast(rz[:], zs[:])
                    bcb = small.tile([DH, 512], f32, tag="bcb", name="bcb")
                    nc.gpsimd.partition_broadcast(bcb[:], rz[:])
                    nc.vector.tensor_mul(
                        aT_sb[pb : pb + DH, m, isl], o_ps[0:DH, :], bcb[:]
                    )

                def wo_tile(i):
                    ps = aux_pool.tile([P, DIM], f32, tag="aux", name="p3_ps")
                    for m in range(2):
                        nc.tensor.matmul(
                            ps[:],
                            aT_sb[:, m, i * P : (i + 1) * P],
                            wo_sb[:, m, :],
                            start=(m == 0),
                            stop=(m == 1),
                        )
                    ost = ostp.tile([P, DIM], f32, tag="ost", name="ost")
                    nc.vector.tensor_copy(ost[:], ps[:])
                    orows = out_d[i * P : (i + 1) * P, :]
                    if i >= 12:
                        # tail tiles: split across both queues to halve the
                        # last transfer on the critical path
                        nc.sync.dma_start(orows[:, 0:256], ost[:, 0:256])
                        nc.gpsimd.dma_start(orows[:, 256:512], ost[:, 256:512])
                    else:
                        eng = nc.sync if i % 2 == 0 else nc.gpsimd
                        eng.dma_start(orows, ost[:])

                def wotile(i):
                    return lambda: wo_tile(i)

                def wo_proj(i4):
                    for ii in range(4):
                        wo_tile(i4 * 4 + ii)

                def kchunk(m, c):
                    return lambda: k_proj(m, c)

                def kchunk0_hi():
                    k_proj(0, 0, 256, 512)

                def qchunk(m, c):
                    return lambda: q_proj(m, c)

                def vpair(g):
                    def f():
                        for jo in (2 * g, 2 * g + 1):
                            ps = aux_pool.tile([P, HC], f32, tag="aux", name="ps_v")
                            for ko in range(KO):
                                nc.tensor.matmul(
                                    ps[:],
                                    ct_sb[:, ko, jo * P : (jo + 1) * P],
                                    wv_sb[:, ko, :],
                                    start=(ko == 0),
                                    stop=(ko == KO - 1),
                                )
                            nc.vector.tensor_copy(
                                vp_sb[:, jo, :, 0:DH],
                                ps[:].rearrange("p (h d) -> p h d", h=4),
                            )
                            if jo < JF:
                                nc.vector.tensor_copy(
                                    vp8_sb[:, jo, :, 0:DH],
                                    ps[:].rearrange("p (h d) -> p h d", h=4),
                                )
                    return f

                # m=0 blocks run one i4 ahead of m=1; projections drip in as
                # per-group fillers, spread thin across blocks 0-13 so early
                # blocks never starve the exp stream (each filler is ~1.5us
                # of PE).  Deadlines: k(m,c) before the first m-block's group
                # 2c; q(m,c) before block (i4=c, m).
                # Fast start: q_proj full chunk + a mini k-proj covering just
                # the first two j-tiles lets exp(0) begin ~8us earlier; the
                # matmuls interleave per-ko so each waits only on its own DMA
                # piece.  The rest of k chunk 0 and the V pairs drip in as
                # block-0 fillers after each exp is issued.
                ps_q = aux_pool.tile([P, 512], f32, tag="aux", name="ps_q")
                ps_k = aux_pool.tile([P, 512], f32, tag="aux", name="ps_k")
                for ko in range(KO):
                    nc.tensor.matmul(
                        ps_q[:], wq_sb[:, ko, 0:P], xt_sb[:, ko, 0:512],
                        start=(ko == 0), stop=(ko == KO - 1),
                    )
                    nc.tensor.matmul(
                        ps_k[:, 0:256], wk_sb[:, ko, 0:P], ct_sb[:, ko, 0:256],
                        start=(ko == 0), stop=(ko == KO - 1),
                    )
                nc.vector.tensor_copy(q8_sb[:, 0, 0, 0:512], ps_q[:])
                nc.vector.tensor_tensor(
                    q8_sb[:, 0, 1, 0:512], ps_q[:], q8_sb[:, 0, 0, 0:512], sub_op
                )
                nc.vector.tensor_copy(k8_sb[:, 0, 0, 0:256], ps_k[:, 0:256])
                nc.vector.tensor_copy(k8_sb[:, 0, 1, 0:256], ps_k[:, 0:256])
                att_block(0, 0, 0, {
                    0: [kchunk0_hi, vpair(0), vpair(1)],
                    1: [kchunk(0, 1), vpair(2)],
                    2: [vpair(3)],
                    3: [kchunk(0, 2), vpair(4)],
                    4: [vpair(5)],
                    5: [kchunk(0, 3), vpair(6)],
                    6: [vpair(7)],
                })
                att_block(0, 0, 1, {0: [qchunk(0, 1)]})
                att_block(1, 0, 0, {0: [kchunk(1, 0)]})
                att_block(1, 0, 1, {0: [kchunk(1, 1)], 4: [qchunk(1, 0)]})
                att_block(0, 1, 0, {1: [kchunk(1, 2)], 3: [kchunk(1, 3)]})
                att_block(0, 1, 1, {0: [qchunk(0, 2)]})
                att_block(2, 0, 0, {
                    0: [wotile(0)], 2: [wotile(1)], 4: [wotile(2)], 6: [wotile(3)],
                })
                att_block(2, 0, 1, {0: [qchunk(1, 1)]})
                att_block(1, 1, 0)
                att_block(1, 1, 1, {0: [qchunk(0, 3)]})
                att_block(3, 0, 0, {
                    0: [wotile(4)], 2: [wotile(5)], 4: [wotile(6)], 6: [wotile(7)],
                })
                att_block(3, 0, 1, {0: [qchunk(1, 2)]})
                att_block(2, 1, 0)
                att_block(2, 1, 1, {0: [qchunk(1, 3)]})
                att_block(3, 1, 0, {
                    0: [wotile(8)], 2: [wotile(9)], 4: [wotile(10)], 6: [wotile(11)],
                })
                att_block(3, 1, 1)
                wo_proj(3)

    nc.compile()
    names = dict(
        xT=xT_d.name,
        cT=cT_d.name,
        wq=wq_d.name,
        wk=wk_d.name,
        wv=wv_d.name,
        wo=wo_d.name,
        out=out_d.name,
    )
    return nc, names


def _get_built():
    if "nc" not in _CACHE:
        _CACHE["nc"], _CACHE["names"] = _build()
    return _CACHE["nc"], _CACHE["names"]


def run(x, context, Wq, Wk, Wv, Wo, bo, trace=False):
    from concourse.bass_utils import run_bass_kernel_spmd

    nc, names = _get_built()
    bf16 = ml_dtypes.bfloat16

    x = np.asarray(x, dtype=np.float32)
    context = np.asarray(context, dtype=np.float32)
    Wq = np.asarray(Wq, dtype=np.float32)
    Wk = np.asarray(Wk, dtype=np.float32)
    Wv = np.asarray(Wv, dtype=np.float32)
    Wo = np.asarray(Wo, dtype=np.float32)
    bo = np.asarray(bo, dtype=np.float32)

    in_maps = []
    for c in range(8):
        b, g = divmod(c, 2)
        cols = slice(g * HC, (g + 1) * HC)
        in_maps.append(
            {
                names["xT"]: np.ascontiguousarray(x[b].T).astype(bf16),
                names["cT"]: np.ascontiguousarray(context[b].T).astype(bf16),
                names["wq"]: np.ascontiguousarray(Wq[:, cols]).astype(bf16),
                names["wk"]: np.ascontiguousarray(Wk[:, cols]).astype(bf16),
                names["wv"]: np.ascontiguousarray(Wv[:, cols]).astype(bf16),
                names["wo"]: np.ascontiguousarray(Wo[cols, :]).astype(bf16),
            }
        )

    res = run_bass_kernel_spmd(
        nc, in_maps, core_ids=list(range(8)), trace=trace,
        stitch_traces=trace,
    )
    out = np.empty((B, N, DIM), dtype=np.float32)
    for b in range(B):
        out[b] = res.results[2 * b][names["out"]] + res.results[2 * b + 1][names["out"]]
    out += bo[None, None, :]
    return out, res


def kernel(x, context, Wq, Wk, Wv, Wo, bo):
    out, _ = run(x, context, Wq, Wk, Wv, Wo, bo, trace=False)
    return out


# revision 44
# speedup vs baseline: 1.0116x; 1.0116x over previous
"""Cross-frame attention kernel for 8 TRN2 NeuronCores.

Sharding: core c handles batch b = c//2 and head-group g = c%2 (4 of the 8
heads).  The host pre-transposes x[b]/context[b] (feature dim onto SBUF
partitions) and casts to bf16; each core computes a partial output
(its 4 heads pushed through the matching Wo rows) and the host sums the
two partials per batch plus the bias.

Device math per core (S^T layout, softmax over the partition j-dim):
  QT = Wq_g^T x^T          [256, 2048] -> fp8 q8 + fp8 residual qr8
  KT = Wk_g^T c^T          [256, 2048] -> fp8 k8 (stored twice, DR slots)
  V  = c Wv_g              [2048, 256] bf16 (+ ones column per head)
  S^T = K_h Q_h^T          via fp8 DoubleRow matmul: k8.T@(q8+qr8)
  exp via ScalarE (scale=1/8 fused), bf16
  O~^T | Z = [V_h|1]^T expS^T   (PSUM accumulate over j, bf16)
  A^T = O~^T * bcast(1/Z)  (reciprocal_approx_fast + gpsimd broadcast)
  out_partial = A^T^T Wo_g [2048, 512] fp32

Logits are |S/8| <~ 1.1 for this problem's scale, so softmax without
max-subtraction is exact in fp32.  fp8 DoubleRow S with the q-side
residual keeps q at ~16-bit precision; only k's fp8 rounding (~2.4%
elementwise) reaches the logits, measured end-to-end rel err ~5.9e-3.

Note: on this machine fp8/DoubleRow matmuls STREAM no faster than bf16
(~0.74 ns/col at N=512 regardless of dtype), but the fp8 S-path lowers
tensor-engine power enough that the chip stays out of DVFS throttling
(which otherwise halves the PE clock after ~100us of sustained bf16
load) — that is where most of the speedup comes from.
"""

import numpy as np
import ml_dtypes

B = 4
N = 2048  # query length
M = 2048  # context length
DIM = 512
HEADS = 8
DH = 64
HC = 256  # head columns handled per core (4 heads)
P = 128
KO = DIM // P  # 4 k-chunks
NI4 = N // 512  # 4 i-chunks of 512
NJ = M // P  # 16 j-chunks
JPG = 2  # j-chunks per exp group (PSUM banks per S^T buffer)
JF = 4  # leading j-chunks per block on the fp8 DoubleRow O path

_CACHE = {}


def _build():
    from contextlib import ExitStack

    import concourse.mybir as mybir
    import concourse.tile as tile
    from concourse import bacc

    bf = mybir.dt.bfloat16
    f32 = mybir.dt.float32
    f8 = mybir.dt.float8e4
    Exp = mybir.ActivationFunctionType.Exp
    DR = mybir.MatmulPerfMode.DoubleRow
    sub_op = mybir.AluOpType.subtract

    nc = bacc.Bacc(None, target_bir_lowering=False, debug=False)
    with tile.TileContext(nc) as tc:
        with ExitStack() as ctx:
            dram = ctx.enter_context(tc.tile_pool(name="dram", bufs=1, space="DRAM"))
            xT_d = dram.tile([DIM, N], bf, kind="ExternalInput")
            cT_d = dram.tile([DIM, M], bf, kind="ExternalInput")
            wq_d = dram.tile([DIM, HC], bf, kind="ExternalInput")
            wk_d = dram.tile([DIM, HC], bf, kind="ExternalInput")
            wv_d = dram.tile([DIM, HC], bf, kind="ExternalInput")
            wo_d = dram.tile([HC, DIM], bf, kind="ExternalInput")
            out_d = dram.tile([N, DIM], f32, kind="ExternalOutput")

            const = ctx.enter_context(tc.tile_pool(name="const", bufs=1))

            xt_sb = const.tile([P, KO, N], bf, tag="xt")
            ct_sb = const.tile([P, KO, M], bf, tag="ct")
            wq_sb = const.tile([P, KO, HC], bf, tag="wq")
            wk_sb = const.tile([P, KO, HC], bf, tag="wk")
            wv_sb = const.tile([P, KO, HC], bf, tag="wv")
            wo_sb = const.tile([P, 2, DIM], bf, tag="wo")
            # fp8 Q (slot 0) + residual (slot 1); head hl at partitions 64*hl
            q8_sb = const.tile([P, 2, 2, N], f8, tag="q8")
            # fp8 K duplicated into both DoubleRow slots
            k8_sb = const.tile([P, 2, 2, M], f8, tag="k8")
            # all 4 heads' V with a trailing ones column: [j, jo, head, 65]
            vp_sb = const.tile([P, NJ, 4, DH + 1], bf, tag="vp")
            # fp8 copy of the first JF j-tiles for the DoubleRow O path.
            # Padded to 68 columns per (jo, head): the DR ldweights outer
            # step must be 16B-aligned (4 heads x 68 = 272 = 16*17).
            vp8_sb = const.tile([P, max(JF, 1), 4, DH + 4], f8, tag="vp8")
            aT_sb = const.tile([P, 2, N], bf, tag="aT")
            ones_sb = const.tile([1, DH], bf, tag="ones1")

            dummy_sb = const.tile([1, 1], f32, tag="dummy")
            nc.vector.memset(ones_sb[:], 1.0)
            nc.vector.memset(vp_sb[:, :, :, DH : DH + 1], 1.0)
            nc.vector.memset(vp8_sb[:], 0.0)
            nc.vector.memset(vp8_sb[:, :, :, DH : DH + 1], 1.0)
            # hoist the exp ACT-table load out of the critical path
            nc.scalar.activation(dummy_sb[:], ones_sb[0:1, 0:1], Exp, scale=1.0)

            # DMA in dependency order.  The critical path to the first exp is
            # q_proj (wq + xt chunk 0) then a mini k-proj over the first two
            # j-tiles (wk + ct[:, :, 0:256]).  A single 128KB piece takes
            # ~6.5us on one DMA engine, so the chunk-0 pieces are split into
            # 256-col halves and the Q-side (sync queue) and K-side (gpsimd
            # queue) streams issue in parallel.
            cT_r = cT_d[:].rearrange("(ko p) i -> p ko i", p=P)
            xT_r = xT_d[:].rearrange("(ko p) i -> p ko i", p=P)
            nc.sync.dma_start(wq_sb[:], wq_d[:].rearrange("(ko p) m -> p ko m", p=P))
            nc.gpsimd.dma_start(wk_sb[:], wk_d[:].rearrange("(ko p) m -> p ko m", p=P))
            # chunk-0 pieces fan out over three DMA-capable queues (Act is
            # idle here) so issue and transfer both parallelize
            for ko in range(KO):
                nc.sync.dma_start(xt_sb[:, ko, 0:256], xT_r[:, ko, 0:256])
                nc.scalar.dma_start(xt_sb[:, ko, 256:512], xT_r[:, ko, 256:512])
            for ko in range(KO):
                nc.gpsimd.dma_start(ct_sb[:, ko, 0:256], cT_r[:, ko, 0:256])
            for ko in range(KO):
                nc.gpsimd.dma_start(ct_sb[:, ko, 256:512], cT_r[:, ko, 256:512])
            nc.gpsimd.dma_start(wv_sb[:], wv_d[:].rearrange("(ko p) m -> p ko m", p=P))
            for i4 in range(1, NI4):
                isl = slice(i4 * 512, (i4 + 1) * 512)
                for ko in range(KO):
                    nc.gpsimd.dma_start(ct_sb[:, ko, isl], cT_r[:, ko, isl])
            for i4 in range(1, NI4):
                isl = slice(i4 * 512, (i4 + 1) * 512)
                for ko in range(KO):
                    nc.sync.dma_start(xt_sb[:, ko, isl], xT_r[:, ko, isl])
            nc.gpsimd.dma_start(wo_sb[:], wo_d[:].rearrange("(r p) n -> p r n", p=P))

            # Single shared PSUM budget (8 banks):
            #   s-tag 2x2 + aux 2x1 (shared with wo) + o 2x1
            with (
                tc.tile_pool(name="s_ps", bufs=2, space="PSUM") as s_pool,
                tc.tile_pool(name="aux_ps", bufs=2, space="PSUM") as aux_pool,
                tc.tile_pool(name="o_ps", bufs=2, space="PSUM") as o_pool,
                tc.tile_pool(name="e_sb", bufs=4) as e_pool,
                tc.tile_pool(name="small", bufs=2) as small,
                tc.tile_pool(name="ost", bufs=2) as ostp,
            ):
                def k_proj(m, chunk, lo=0, hi=512):
                    isl = slice(chunk * 512 + lo, chunk * 512 + hi)
                    ps = aux_pool.tile([P, 512], f32, tag="aux", name="ps_k")
                    for ko in range(KO):
                        nc.tensor.matmul(
                            ps[:, 0 : hi - lo],
                            wk_sb[:, ko, m * P : (m + 1) * P],
                            ct_sb[:, ko, isl],
                            start=(ko == 0),
                            stop=(ko == KO - 1),
                        )
                    nc.vector.tensor_copy(k8_sb[:, m, 0, isl], ps[:, 0 : hi - lo])
                    nc.vector.tensor_copy(k8_sb[:, m, 1, isl], ps[:, 0 : hi - lo])

                def q_proj(m, chunk):
                    isl = slice(chunk * 512, (chunk + 1) * 512)
                    ps = aux_pool.tile([P, 512], f32, tag="aux", name="ps_q")
                    for ko in range(KO):
                        nc.tensor.matmul(
                            ps[:],
                            wq_sb[:, ko, m * P : (m + 1) * P],
                            xt_sb[:, ko, isl],
                            start=(ko == 0),
                            stop=(ko == KO - 1),
                        )
                    nc.vector.tensor_copy(q8_sb[:, m, 0, isl], ps[:])
                    nc.vector.tensor_tensor(
                        q8_sb[:, m, 1, isl], ps[:], q8_sb[:, m, 0, isl], sub_op
                    )

                def att_block(i4, m, hl, fillers=None, vfill=False):
                    isl = slice(i4 * 512, (i4 + 1) * 512)
                    h = 2 * m + hl
                    pb = DH * hl
                    # 68 partitions: rows 65-67 take the vp8 padding columns
                    # (zeros); Z stays at row DH
                    o_ps = o_pool.tile([DH + 4, 512], f32, tag="o", name="o_ps")
                    for jg in range(NJ // JPG):
                        s_ps = s_pool.tile([P, JPG, 512], f32, tag="s", name="s_ps")
                        for jj in range(JPG):
                            j = jg * JPG + jj
                            nc.tensor.matmul(
                                s_ps[:, jj, :],
                                k8_sb[pb : pb + DH, m, :, j * P : (j + 1) * P],
                                q8_sb[pb : pb + DH, m, :, isl],
                                start=True,
                                stop=True,
                                perf_mode=DR,
                            )
                        dr8 = jg < JF // JPG
                        if dr8:
                            e_sb = e_pool.tile([P, JPG, 512], f8, tag="e8",
                                               name="e8_sb")
                        else:
                            e_sb = e_pool.tile([P, JPG, 512], bf, tag="e",
                                               name="e_sb")
                        nc.scalar.activation(e_sb[:], s_ps[:], Exp, scale=0.125)
                        # projection fillers sit between the S matmuls and the
                        # O accumulation so they overlap the act-engine exp.
                        if vfill:
                            vpair(jg)()
                        for f in (fillers or {}).get(jg, []):
                            f()
                        if dr8:
                            # fp8 DoubleRow: both j-tiles of the group in one
                            # matmul (slots carry the pair) — half the PE time
                            nc.tensor.matmul(
                                o_ps[:],
                                vp8_sb[:, jg * JPG : jg * JPG + JPG, h, :],
                                e_sb[:],
                                start=(jg == 0),
                                stop=False,
                                perf_mode=DR,
                            )
                        else:
                            for jj in range(JPG):
                                j = jg * JPG + jj
                                nc.tensor.matmul(
                                    o_ps[0 : DH + 1, :],
                                    vp_sb[:, j, h, :],
                                    e_sb[:, jj, :],
                                    start=(j == 0),
                                    stop=(j == NJ - 1),
                                )
                    # recip_approx_fast's fp32 bit-trick misreads PSUM on HW:
                    # stage Z into SBUF first.
                    zs = small.tile([1, 512], f32, tag="zs", name="zs")
                    nc.vector.tensor_copy(zs[:], o_ps[DH : DH + 1, :])
                    rz = small.tile([1, 512], f32, tag="rz", name="rz")
                    nc.vector.reciprocal_approx_fast(rz[:], zs[:])
                    bcb = small.tile([DH, 512], f32, tag="bcb", name="bcb")
                    nc.gpsimd.partition_broadcast(bcb[:], rz[:])
                    nc.vector.tensor_mul(
                        aT_sb[pb : pb + DH, m, isl], o_ps[0:DH, :], bcb[:]
                    )

                def wo_tile(i):
                    ps = aux_pool.tile([P, DIM], f32, tag="aux", name="p3_ps")
                    for m in range(2):
                        nc.tensor.matmul(
                            ps[:],
                            aT_sb[:, m, i * P : (i + 1) * P],
                            wo_sb[:, m, :],
                            start=(m == 0),
                            stop=(m == 1),
                        )
                    ost = ostp.tile([P, DIM], f32, tag="ost", name="ost")
                    nc.vector.tensor_copy(ost[:], ps[:])
                    orows = out_d[i * P : (i + 1) * P, :]
                    if i >= 12:
                        # tail tiles: split across both queues to halve the
                        # last transfer on the critical path
                        nc.sync.dma_start(orows[:, 0:256], ost[:, 0:256])
                        nc.gpsimd.dma_start(orows[:, 256:512], ost[:, 256:512])
                    else:
                        eng = nc.sync if i % 2 == 0 else nc.gpsimd
                        eng.dma_start(orows, ost[:])

                def wotile(i):
                    return lambda: wo_tile(i)

                def wo_proj(i4):
                    for ii in range(4):
                        wo_tile(i4 * 4 + ii)

                def kchunk(m, c):
                    return lambda: k_proj(m, c)

                def kchunk0_hi():
                    k_proj(0, 0, 256, 512)

                def qchunk(m, c):
                    return lambda: q_proj(m, c)

                def vpair(g):
                    def f():
                        for jo in (2 * g, 2 * g + 1):
                            ps = aux_pool.tile([P, HC], f32, tag="aux", name="ps_v")
                            for ko in range(KO):
                                nc.tensor.matmul(
                                    ps[:],
                                    ct_sb[:, ko, jo * P : (jo + 1) * P],
                                    wv_sb[:, ko, :],
                                    start=(ko == 0),
                                    stop=(ko == KO - 1),
                                )
                            nc.vector.tensor_copy(
                                vp_sb[:, jo, :, 0:DH],
                                ps[:].rearrange("p (h d) -> p h d", h=4),
                            )
                            if jo < JF:
                                nc.vector.tensor_copy(
                                    vp8_sb[:, jo, :, 0:DH],
                                    ps[:].rearrange("p (h d) -> p h d", h=4),
                                )
                    return f

                # m=0 blocks run one i4 ahead of m=1; projections drip in as
                # per-group fillers, spread thin across blocks 0-13 so early
                # blocks never starve the exp stream (each filler is ~1.5us
                # of PE).  Deadlines: k(m,c) before the first m-block's group
                # 2c; q(m,c) before block (i4=c, m).
                # Fast start: q_proj full chunk + a mini k-proj covering just
                # the first two j-tiles lets exp(0) begin ~8us earlier; the
                # matmuls interleave per-ko so each waits only on its own DMA
                # piece.  The rest of k chunk 0 and the V pairs drip in as
                # block-0 fillers after each exp is issued.
                ps_q = aux_pool.tile([P, 512], f32, tag="aux", name="ps_q")
                ps_k = aux_pool.tile([P, 512], f32, tag="aux", name="ps_k")
                for ko in range(KO):
                    nc.tensor.matmul(
                        ps_q[:], wq_sb[:, ko, 0:P], xt_sb[:, ko, 0:512],
                        start=(ko == 0), stop=(ko == KO - 1),
                    )
                    nc.tensor.matmul(
                        ps_k[:, 0:256], wk_sb[:, ko, 0:P], ct_sb[:, ko, 0:256],
                        start=(ko == 0), stop=(ko == KO - 1),
                    )
                nc.vector.tensor_copy(q8_sb[:, 0, 0, 0:512], ps_q[:])
                nc.vector.tensor_tensor(
                    q8_sb[:, 0, 1, 0:512], ps_q[:], q8_sb[:, 0, 0, 0:512], sub_op
                )
                nc.vector.tensor_copy(k8_sb[:, 0, 0, 0:256], ps_k[:, 0:256])
                nc.vector.tensor_copy(k8_sb[:, 0, 1, 0:256], ps_k[:, 0:256])
                att_block(0, 0, 0, {
                    0: [kchunk0_hi, vpair(0), vpair(1)],
                    1: [kchunk(0, 1), vpair(2)],
                    2: [vpair(3)],
                    3: [kchunk(0, 2), vpair(4)],
                    4: [vpair(5)],
                    5: [kchunk(0, 3), vpair(6)],
                    6: [vpair(7)],
                })
                att_block(0, 0, 1, {0: [qchunk(0, 1)]})
                att_block(1, 0, 0, {0: [kchunk(1, 0)]})
                att_block(1, 0, 1, {0: [kchunk(1, 1)], 4: [qchunk(1, 0)]})
                att_block(0, 1, 0, {1: [kchunk(1, 2)], 3: [kchunk(1, 3)]})
                att_block(0, 1, 1, {0: [qchunk(0, 2)]})
                att_block(2, 0, 0, {
                    0: [wotile(0)], 2: [wotile(1)], 4: [wotile(2)], 6: [wotile(3)],
                })
                att_block(2, 0, 1, {0: [qchunk(1, 1)]})
                att_block(1, 1, 0)
                att_block(1, 1, 1, {0: [qchunk(0, 3)]})
                att_block(3, 0, 0, {
                    0: [wotile(4)], 2: [wotile(5)], 4: [wotile(6)], 6: [wotile(7)],
                })
                att_block(3, 0, 1, {0: [qchunk(1, 2)]})
                att_block(2, 1, 0)
                att_block(2, 1, 1, {0: [qchunk(1, 3)]})
                att_block(3, 1, 0, {
                    0: [wotile(8)], 2: [wotile(9)], 4: [wotile(10)], 6: [wotile(11)],
                })
                att_block(3, 1, 1)
                wo_proj(3)

    nc.compile()
    names = dict(
        xT=xT_d.name,
        cT=cT_d.name,
        wq=wq_d.name,
        wk=wk_d.name,
        wv=wv_d.name,
        wo=wo_d.name,
        out=out_d.name,
    )
    return nc, names


def _get_built():
    if "nc" not in _CACHE:
        _CACHE["nc"], _CACHE["names"] = _build()
    return _CACHE["nc"], _CACHE["names"]


def run(x, context, Wq, Wk, Wv, Wo, bo, trace=False):
    from concourse.bass_utils import run_bass_kernel_spmd

    nc, names = _get_built()
    bf16 = ml_dtypes.bfloat16

    x = np.asarray(x, dtype=np.float32)
    context = np.asarray(context, dtype=np.float32)
    Wq = np.asarray(Wq, dtype=np.float32)
    Wk = np.asarray(Wk, dtype=np.float32)
    Wv = np.asarray(Wv, dtype=np.float32)
    Wo = np.asarray(Wo, dtype=np.float32)
    bo = np.asarray(bo, dtype=np.float32)

    in_maps = []
    for c in range(8):
        b, g = divmod(c, 2)
        cols = slice(g * HC, (g + 1) * HC)
        in_maps.append(
            {
                names["xT"]: np.ascontiguousarray(x[b].T).astype(bf16),
                names["cT"]: np.ascontiguousarray(context[b].T).astype(bf16),
                names["wq"]: np.ascontiguousarray(Wq[:, cols]).astype(bf16),
                names["wk"]: np.ascontiguousarray(Wk[:, cols]).astype(bf16),
                names["wv"]: np.ascontiguousarray(Wv[:, cols]).astype(bf16),
                names["wo"]: np.ascontiguousarray(Wo[cols, :]).astype(bf16),
            }
        )

    res = run_bass_kernel_spmd(
        nc, in_maps, core_ids=list(range(8)), trace=trace,
        stitch_traces=trace,
    )
    out = np.empty((B, N, DIM), dtype=np.float32)
    for b in range(B):
        out[b] = res.results[2 * b][names["out"]] + res.results[2 * b + 1][names["out"]]
    out += bo[None, None, :]
    return out, res


def kernel(x, context, Wq, Wk, Wv, Wo, bo):
    out, _ = run(x, context, Wq, Wk, Wv, Wo, bo, trace=False)
    return out


# revision 45
# speedup vs baseline: 1.0123x; 1.0007x over previous
"""Cross-frame attention kernel for 8 TRN2 NeuronCores.

Sharding: core c handles batch b = c//2 and head-group g = c%2 (4 of the 8
heads).  The host pre-transposes x[b]/context[b] (feature dim onto SBUF
partitions) and casts to bf16; each core computes a partial output
(its 4 heads pushed through the matching Wo rows) and the host sums the
two partials per batch plus the bias.

Device math per core (S^T layout, softmax over the partition j-dim):
  QT = Wq_g^T x^T          [256, 2048] -> fp8 q8 + fp8 residual qr8
  KT = Wk_g^T c^T          [256, 2048] -> fp8 k8 (stored twice, DR slots)
  V  = c Wv_g              [2048, 256] bf16 (+ ones column per head)
  S^T = K_h Q_h^T          via fp8 DoubleRow matmul: k8.T@(q8+qr8)
  exp via ScalarE (scale=1/8 fused), bf16 (fp8 for the first JF j-tiles)
  O~^T | Z = [V_h|1]^T expS^T   (PSUM accumulate over j; the first JF
           j-tiles per block use fp8 DoubleRow with the j-pair packed
           into the slots — half the matmuls for that span)
  A^T = O~^T * bcast(1/Z)  (reciprocal_approx_fast + gpsimd broadcast)
  out_partial = A^T^T Wo_g [2048, 512] fp32

Logits are |S/8| <~ 1.1 for this problem's scale, so softmax without
max-subtraction is exact in fp32.  fp8 DoubleRow S with the q-side
residual keeps q at ~16-bit precision; only k's fp8 rounding (~2.4%
elementwise) reaches the logits.  The JF=4 fp8 O-span adds e/v fp8
noise over 1/4 of the softmax sum; measured end-to-end rel err 1.55e-2
(gate 2e-2, deterministic).  Set JF=0 for the conservative 5.9e-3
variant (~1% slower).

Note: on this machine fp8/DoubleRow matmuls STREAM no faster than bf16
(~0.74 ns/col at N=512 regardless of dtype), but the fp8 S-path lowers
tensor-engine power enough that the chip stays out of DVFS throttling
(which otherwise halves the PE clock after ~100us of sustained bf16
load) — that is where most of the speedup comes from.
"""

import numpy as np
import ml_dtypes

B = 4
N = 2048  # query length
M = 2048  # context length
DIM = 512
HEADS = 8
DH = 64
HC = 256  # head columns handled per core (4 heads)
P = 128
KO = DIM // P  # 4 k-chunks
NI4 = N // 512  # 4 i-chunks of 512
NJ = M // P  # 16 j-chunks
JPG = 2  # j-chunks per exp group (PSUM banks per S^T buffer)
JF = 4  # leading j-chunks per block on the fp8 DoubleRow O path

_CACHE = {}


def _build():
    from contextlib import ExitStack

    import concourse.mybir as mybir
    import concourse.tile as tile
    from concourse import bacc

    bf = mybir.dt.bfloat16
    f32 = mybir.dt.float32
    f8 = mybir.dt.float8e4
    Exp = mybir.ActivationFunctionType.Exp
    DR = mybir.MatmulPerfMode.DoubleRow
    sub_op = mybir.AluOpType.subtract

    nc = bacc.Bacc(None, target_bir_lowering=False, debug=False)
    with tile.TileContext(nc) as tc:
        with ExitStack() as ctx:
            dram = ctx.enter_context(tc.tile_pool(name="dram", bufs=1, space="DRAM"))
            xT_d = dram.tile([DIM, N], bf, kind="ExternalInput")
            cT_d = dram.tile([DIM, M], bf, kind="ExternalInput")
            wq_d = dram.tile([DIM, HC], bf, kind="ExternalInput")
            wk_d = dram.tile([DIM, HC], bf, kind="ExternalInput")
            wv_d = dram.tile([DIM, HC], bf, kind="ExternalInput")
            wo_d = dram.tile([HC, DIM], bf, kind="ExternalInput")
            out_d = dram.tile([N, DIM], f32, kind="ExternalOutput")

            const = ctx.enter_context(tc.tile_pool(name="const", bufs=1))

            xt_sb = const.tile([P, KO, N], bf, tag="xt")
            ct_sb = const.tile([P, KO, M], bf, tag="ct")
            wq_sb = const.tile([P, KO, HC], bf, tag="wq")
            wk_sb = const.tile([P, KO, HC], bf, tag="wk")
            wv_sb = const.tile([P, KO, HC], bf, tag="wv")
            wo_sb = const.tile([P, 2, DIM], bf, tag="wo")
            # fp8 Q (slot 0) + residual (slot 1); head hl at partitions 64*hl
            q8_sb = const.tile([P, 2, 2, N], f8, tag="q8")
            # fp8 K duplicated into both DoubleRow slots
            k8_sb = const.tile([P, 2, 2, M], f8, tag="k8")
            # all 4 heads' V with a trailing ones column: [j, jo, head, 65]
            vp_sb = const.tile([P, NJ, 4, DH + 1], bf, tag="vp")
            # fp8 copy of the first JF j-tiles for the DoubleRow O path.
            # Padded to 68 columns per (jo, head): the DR ldweights outer
            # step must be 16B-aligned (4 heads x 68 = 272 = 16*17).
            vp8_sb = const.tile([P, max(JF, 1), 4, DH + 4], f8, tag="vp8")
            aT_sb = const.tile([P, 2, N], bf, tag="aT")
            ones_sb = const.tile([1, DH], bf, tag="ones1")

            dummy_sb = const.tile([1, 1], f32, tag="dummy")
            nc.vector.memset(ones_sb[:], 1.0)
            nc.vector.memset(vp_sb[:, :, :, DH : DH + 1], 1.0)
            nc.vector.memset(vp8_sb[:], 0.0)
            nc.vector.memset(vp8_sb[:, :, :, DH : DH + 1], 1.0)
            # hoist the exp ACT-table load out of the critical path
            nc.scalar.activation(dummy_sb[:], ones_sb[0:1, 0:1], Exp, scale=1.0)

            # DMA in dependency order.  The critical path to the first exp is
            # q_proj (wq + xt chunk 0) then a mini k-proj over the first two
            # j-tiles (wk + ct[:, :, 0:256]).  A single 128KB piece takes
            # ~6.5us on one DMA engine, so the chunk-0 pieces are split into
            # 256-col halves and the Q-side (sync queue) and K-side (gpsimd
            # queue) streams issue in parallel.
            cT_r = cT_d[:].rearrange("(ko p) i -> p ko i", p=P)
            xT_r = xT_d[:].rearrange("(ko p) i -> p ko i", p=P)
            nc.sync.dma_start(wq_sb[:], wq_d[:].rearrange("(ko p) m -> p ko m", p=P))
            nc.gpsimd.dma_start(wk_sb[:], wk_d[:].rearrange("(ko p) m -> p ko m", p=P))
            # chunk-0 pieces fan out over three DMA-capable queues (Act is
            # idle here) so issue and transfer both parallelize
            for ko in range(KO):
                nc.sync.dma_start(xt_sb[:, ko, 0:256], xT_r[:, ko, 0:256])
                nc.scalar.dma_start(xt_sb[:, ko, 256:512], xT_r[:, ko, 256:512])
            for ko in range(KO):
                nc.gpsimd.dma_start(ct_sb[:, ko, 0:256], cT_r[:, ko, 0:256])
            for ko in range(KO):
                nc.gpsimd.dma_start(ct_sb[:, ko, 256:512], cT_r[:, ko, 256:512])
            nc.gpsimd.dma_start(wv_sb[:], wv_d[:].rearrange("(ko p) m -> p ko m", p=P))
            for i4 in range(1, NI4):
                isl = slice(i4 * 512, (i4 + 1) * 512)
                for ko in range(KO):
                    nc.gpsimd.dma_start(ct_sb[:, ko, isl], cT_r[:, ko, isl])
            for i4 in range(1, NI4):
                isl = slice(i4 * 512, (i4 + 1) * 512)
                for ko in range(KO):
                    nc.sync.dma_start(xt_sb[:, ko, isl], xT_r[:, ko, isl])
            nc.gpsimd.dma_start(wo_sb[:], wo_d[:].rearrange("(r p) n -> p r n", p=P))

            # Single shared PSUM budget (8 banks):
            #   s-tag 2x2 + aux 2x1 (shared with wo) + o 2x1
            with (
                tc.tile_pool(name="s_ps", bufs=2, space="PSUM") as s_pool,
                tc.tile_pool(name="aux_ps", bufs=2, space="PSUM") as aux_pool,
                tc.tile_pool(name="o_ps", bufs=2, space="PSUM") as o_pool,
                tc.tile_pool(name="e_sb", bufs=4) as e_pool,
                tc.tile_pool(name="small", bufs=2) as small,
                tc.tile_pool(name="ost", bufs=2) as ostp,
            ):
                def k_proj(m, chunk, lo=0, hi=512):
                    isl = slice(chunk * 512 + lo, chunk * 512 + hi)
                    ps = aux_pool.tile([P, 512], f32, tag="aux", name="ps_k")
                    for ko in range(KO):
                        nc.tensor.matmul(
                            ps[:, 0 : hi - lo],
                            wk_sb[:, ko, m * P : (m + 1) * P],
                            ct_sb[:, ko, isl],
                            start=(ko == 0),
                            stop=(ko == KO - 1),
                        )
                    nc.vector.tensor_copy(k8_sb[:, m, 0, isl], ps[:, 0 : hi - lo])
                    nc.vector.tensor_copy(k8_sb[:, m, 1, isl], ps[:, 0 : hi - lo])

                def q_proj(m, chunk):
                    isl = slice(chunk * 512, (chunk + 1) * 512)
                    ps = aux_pool.tile([P, 512], f32, tag="aux", name="ps_q")
                    for ko in range(KO):
                        nc.tensor.matmul(
                            ps[:],
                            wq_sb[:, ko, m * P : (m + 1) * P],
                            xt_sb[:, ko, isl],
                            start=(ko == 0),
                            stop=(ko == KO - 1),
                        )
                    nc.vector.tensor_copy(q8_sb[:, m, 0, isl], ps[:])
                    nc.vector.tensor_tensor(
                        q8_sb[:, m, 1, isl], ps[:], q8_sb[:, m, 0, isl], sub_op
                    )

                def att_block(i4, m, hl, fillers=None, vfill=False):
                    isl = slice(i4 * 512, (i4 + 1) * 512)
                    h = 2 * m + hl
                    pb = DH * hl
                    # 68 partitions: rows 65-67 take the vp8 padding columns
                    # (zeros); Z stays at row DH
                    o_ps = o_pool.tile([DH + 4, 512], f32, tag="o", name="o_ps")
                    for jg in range(NJ // JPG):
                        s_ps = s_pool.tile([P, JPG, 512], f32, tag="s", name="s_ps")
                        for jj in range(JPG):
                            j = jg * JPG + jj
                            nc.tensor.matmul(
                                s_ps[:, jj, :],
                                k8_sb[pb : pb + DH, m, :, j * P : (j + 1) * P],
                                q8_sb[pb : pb + DH, m, :, isl],
                                start=True,
                                stop=True,
                                perf_mode=DR,
                            )
                        dr8 = jg < JF // JPG
                        if dr8:
                            e_sb = e_pool.tile([P, JPG, 512], f8, tag="e8",
                                               name="e8_sb")
                        else:
                            e_sb = e_pool.tile([P, JPG, 512], bf, tag="e",
                                               name="e_sb")
                        nc.scalar.activation(e_sb[:], s_ps[:], Exp, scale=0.125)
                        # projection fillers sit between the S matmuls and the
                        # O accumulation so they overlap the act-engine exp.
                        if vfill:
                            vpair(jg)()
                        for f in (fillers or {}).get(jg, []):
                            f()
                        if dr8:
                            # fp8 DoubleRow: both j-tiles of the group in one
                            # matmul (slots carry the pair) — half the PE time
                            nc.tensor.matmul(
                                o_ps[:],
                                vp8_sb[:, jg * JPG : jg * JPG + JPG, h, :],
                                e_sb[:],
                                start=(jg == 0),
                                stop=False,
                                perf_mode=DR,
                            )
                        else:
                            for jj in range(JPG):
                                j = jg * JPG + jj
                                nc.tensor.matmul(
                                    o_ps[0 : DH + 1, :],
                                    vp_sb[:, j, h, :],
                                    e_sb[:, jj, :],
                                    start=(j == 0),
                                    stop=(j == NJ - 1),
                                )
                    # recip_approx_fast's fp32 bit-trick misreads PSUM on HW:
                    # stage Z into SBUF first.
                    zs = small.tile([1, 512], f32, tag="zs", name="zs")
                    nc.vector.tensor_copy(zs[:], o_ps[DH : DH + 1, :])
                    rz = small.tile([1, 512], f32, tag="rz", name="rz")
                    nc.vector.reciprocal_approx_fast(rz[:], zs[:])
                    bcb = small.tile([DH, 512], f32, tag="bcb", name="bcb")
                    nc.gpsimd.partition_broadcast(bcb[:], rz[:])
                    nc.vector.tensor_mul(
                        aT_sb[pb : pb + DH, m, isl], o_ps[0:DH, :], bcb[:]
                    )

                def wo_tile(i):
                    ps = aux_pool.tile([P, DIM], f32, tag="aux", name="p3_ps")
                    for m in range(2):
                        nc.tensor.matmul(
                            ps[:],
                            aT_sb[:, m, i * P : (i + 1) * P],
                            wo_sb[:, m, :],
                            start=(m == 0),
                            stop=(m == 1),
                        )
                    ost = ostp.tile([P, DIM], f32, tag="ost", name="ost")
                    nc.vector.tensor_copy(ost[:], ps[:])
                    orows = out_d[i * P : (i + 1) * P, :]
                    if i >= 12:
                        # tail tiles: split across both queues to halve the
                        # last transfer on the critical path
                        nc.sync.dma_start(orows[:, 0:256], ost[:, 0:256])
                        nc.gpsimd.dma_start(orows[:, 256:512], ost[:, 256:512])
                    else:
                        eng = nc.sync if i % 2 == 0 else nc.gpsimd
                        eng.dma_start(orows, ost[:])

                def wotile(i):
                    return lambda: wo_tile(i)

                def wo_proj(i4):
                    for ii in range(4):
                        wo_tile(i4 * 4 + ii)

                def kchunk(m, c):
                    return lambda: k_proj(m, c)

                def kchunk0_hi():
                    k_proj(0, 0, 256, 512)

                def qchunk(m, c):
                    return lambda: q_proj(m, c)

                def vpair(g):
                    def f():
                        for jo in (2 * g, 2 * g + 1):
                            ps = aux_pool.tile([P, HC], f32, tag="aux", name="ps_v")
                            for ko in range(KO):
                                nc.tensor.matmul(
                                    ps[:],
                                    ct_sb[:, ko, jo * P : (jo + 1) * P],
                                    wv_sb[:, ko, :],
                                    start=(ko == 0),
                                    stop=(ko == KO - 1),
                                )
                            nc.vector.tensor_copy(
                                vp_sb[:, jo, :, 0:DH],
                                ps[:].rearrange("p (h d) -> p h d", h=4),
                            )
                            if jo < JF:
                                nc.vector.tensor_copy(
                                    vp8_sb[:, jo, :, 0:DH],
                                    ps[:].rearrange("p (h d) -> p h d", h=4),
                                )
                    return f

                # m=0 blocks run one i4 ahead of m=1; projections drip in as
                # per-group fillers, spread thin across blocks 0-13 so early
                # blocks never starve the exp stream (each filler is ~1.5us
                # of PE).  Deadlines: k(m,c) before the first m-block's group
                # 2c; q(m,c) before block (i4=c, m).
                # Fast start: q_proj full chunk + a mini k-proj covering just
                # the first two j-tiles lets exp(0) begin ~8us earlier; the
                # matmuls interleave per-ko so each waits only on its own DMA
                # piece.  The rest of k chunk 0 and the V pairs drip in as
                # block-0 fillers after each exp is issued.
                ps_q = aux_pool.tile([P, 512], f32, tag="aux", name="ps_q")
                ps_k = aux_pool.tile([P, 512], f32, tag="aux", name="ps_k")
                for ko in range(KO):
                    nc.tensor.matmul(
                        ps_q[:], wq_sb[:, ko, 0:P], xt_sb[:, ko, 0:512],
                        start=(ko == 0), stop=(ko == KO - 1),
                    )
                    nc.tensor.matmul(
                        ps_k[:, 0:256], wk_sb[:, ko, 0:P], ct_sb[:, ko, 0:256],
                        start=(ko == 0), stop=(ko == KO - 1),
                    )
                nc.vector.tensor_copy(q8_sb[:, 0, 0, 0:512], ps_q[:])
                nc.vector.tensor_tensor(
                    q8_sb[:, 0, 1, 0:512], ps_q[:], q8_sb[:, 0, 0, 0:512], sub_op
                )
                nc.vector.tensor_copy(k8_sb[:, 0, 0, 0:256], ps_k[:, 0:256])
                nc.vector.tensor_copy(k8_sb[:, 0, 1, 0:256], ps_k[:, 0:256])
                att_block(0, 0, 0, {
                    0: [kchunk0_hi, vpair(0), vpair(1)],
                    1: [kchunk(0, 1), vpair(2)],
                    2: [vpair(3)],
                    3: [kchunk(0, 2), vpair(4)],
                    4: [vpair(5)],
                    5: [kchunk(0, 3), vpair(6)],
                    6: [vpair(7)],
                })
                att_block(0, 0, 1, {0: [qchunk(0, 1)]})
                att_block(1, 0, 0, {0: [kchunk(1, 0)]})
                att_block(1, 0, 1, {0: [kchunk(1, 1)], 4: [qchunk(1, 0)]})
                att_block(0, 1, 0, {1: [kchunk(1, 2)], 3: [kchunk(1, 3)]})
                att_block(0, 1, 1, {0: [qchunk(0, 2)]})
                att_block(2, 0, 0, {
                    0: [wotile(0)], 2: [wotile(1)], 4: [wotile(2)], 6: [wotile(3)],
                })
                att_block(2, 0, 1, {0: [qchunk(1, 1)]})
                att_block(1, 1, 0)
                att_block(1, 1, 1, {0: [qchunk(0, 3)]})
                att_block(3, 0, 0, {
                    0: [wotile(4)], 2: [wotile(5)], 4: [wotile(6)], 6: [wotile(7)],
                })
                att_block(3, 0, 1, {0: [qchunk(1, 2)]})
                att_block(2, 1, 0)
                att_block(2, 1, 1, {0: [qchunk(1, 3)]})
                att_block(3, 1, 0, {
                    0: [wotile(8)], 2: [wotile(9)], 4: [wotile(10)], 6: [wotile(11)],
                })
                att_block(3, 1, 1)
                wo_proj(3)

    nc.compile()
    names = dict(
        xT=xT_d.name,
        cT=cT_d.name,
        wq=wq_d.name,
        wk=wk_d.name,
        wv=wv_d.name,
        wo=wo_d.name,
        out=out_d.name,
    )
    return nc, names


def _get_built():
    if "nc" not in _CACHE:
        _CACHE["nc"], _CACHE["names"] = _build()
    return _CACHE["nc"], _CACHE["names"]


def run(x, context, Wq, Wk, Wv, Wo, bo, trace=False):
    from concourse.bass_utils import run_bass_kernel_spmd

    nc, names = _get_built()
    bf16 = ml_dtypes.bfloat16

    x = np.asarray(x, dtype=np.float32)
    context = np.asarray(context, dtype=np.float32)
    Wq = np.asarray(Wq, dtype=np.float32)
    Wk = np.asarray(Wk, dtype=np.float32)
    Wv = np.asarray(Wv, dtype=np.float32)
    Wo = np.asarray(Wo, dtype=np.float32)
    bo = np.asarray(bo, dtype=np.float32)

    in_maps = []
    for c in range(8):
        b, g = divmod(c, 2)
        cols = slice(g * HC, (g + 1) * HC)
        in_maps.append(
            {
                names["xT"]: np.ascontiguousarray(x[b].T).astype(bf16),
                names["cT"]: np.ascontiguousarray(context[b].T).astype(bf16),
                names["wq"]: np.ascontiguousarray(Wq[:, cols]).astype(bf16),
                names["wk"]: np.ascontiguousarray(Wk[:, cols]).astype(bf16),
                names["wv"]: np.ascontiguousarray(Wv[:, cols]).astype(bf16),
                names["wo"]: np.ascontiguousarray(Wo[cols, :]).astype(bf16),
            }
        )

    res = run_bass_kernel_spmd(
        nc, in_maps, core_ids=list(range(8)), trace=trace,
        stitch_traces=trace,
    )
    out = np.empty((B, N, DIM), dtype=np.float32)
    for b in range(B):
        out[b] = res.results[2 * b][names["out"]] + res.results[2 * b + 1][names["out"]]
    out += bo[None, None, :]
    return out, res


def kernel(x, context, Wq, Wk, Wv, Wo, bo):
    out, _ = run(x, context, Wq, Wk, Wv, Wo, bo, trace=False)
    return out
